# revision 37
# baseline (speedup 1.0000x reference)
"""Causal self-attention Trainium2 Bass kernel.

Problem: B=4, T=2048, C=1024, H=16 heads, head_dim=64, fp32.
    qkv = x @ Wqkv + bqkv ; per-head causal softmax attention ; out = attn @ Wo + bo

Sharding (8 NeuronCores): core c -> (batch b = c//2, head-group g = c%2).
Each core computes qkv for its batch restricted to its 8 heads, attention for
those heads, and a partial output projection against its 512 rows of Wo.
The host sums the two partials of each batch pair (the tensor-parallel
all-reduce), adds bo, and stacks batches.

On-core dataflow (matmul dtype MM_DT = fp16 by default; PSUM accumulation is
always fp32):

  The kernel is emitted QUERY-BLOCK-MAJOR so that projection (phase-1) matmul
  work interleaves with attention (phase-2) work on the PE.  Attention
  couples PE->ACT->PE (scores -> exp -> weighted sum), and the exp stream on
  the Scalar engine is slightly slower than the PE's attention work, so a
  pure attention phase starves the PE in sub-microsecond gaps; the PE clock
  monitor then halves the PE clock (K=4/8 gating needs ~3.4us of
  uninterrupted work to re-warm).  Interleaving the independent qkv
  projection matmuls keeps the PE saturated.

    round tc=0:  qT/kT/v chunk 0 (t in [0,512))
    round q:     attention blocks (h, q) for all 8 heads,
                 interleaved with qT/kT/v chunk q+1
    tail:        out_partial[t,c] = attnT-tile^T @ Wo-rows (PSUM-accumulated)

  Attention per (head, 512-query block): S_T[k,q] = kT-tile^T @ qT, exp via
  ACT (scale=1/8 folded in; scores bounded ~|3.2| so no max subtraction;
  full key-tile pairs share one 1024-wide exp), causal masking via a host
  triangular tile + sub-range accumulation, attnT_aug = [v|1]^T @ expS_T
  accumulated over key tiles (row 64 = softmax denominator).  Normalization
  is software-pipelined one block behind: denom row -> SBUF, ones x denom
  broadcast matmul -> PSUM, fast reciprocal -> SBUF, multiply into attnT.
  bq/bk applied as per-partition adds during the PSUM->SBUF copy; bv as a
  K=1 rank-1 matmul update; bo added on host.
"""

import os as _os
import sys

if "/opt/trn_rl_repo" not in sys.path:
    sys.path.insert(0, "/opt/trn_rl_repo")

import numpy as np

import concourse.bass as bass
import concourse.tile as tile
from concourse import bacc, mybir
from concourse.bass_utils import run_bass_kernel_spmd

F32 = mybir.dt.float32
F16 = mybir.dt.float16
BF16 = mybir.dt.bfloat16
EXP = mybir.ActivationFunctionType.Exp

# Matmul operand dtype (must be 2-byte: 1 PE cycle/row and fits SBUF budget)
MM_DT = {"f16": F16, "bf16": BF16}[_os.environ.get("MM_DT", "f16")]

B, T, C = 4, 2048, 1024
H, D = 16, 64
HPC = 8          # heads per core
HD = HPC * D     # 512: per-core head-dim slab
N_CORES = 8
SCALE = D ** -0.5

KO = C // 128        # 8 contraction tiles over C
TC = T // 512        # 4 t-chunks of 512
NQ = T // 512        # 4 query blocks per head
NKT = T // 128       # 16 key tiles
HDO = HD // 128      # 4 hd tiles


def _np_of(dt):
    return np.dtype(mybir.dt.np(dt))


def build_nc(mm_dt=None, use_bias=True):
    mm_dt = mm_dt or MM_DT
    nc = bacc.Bacc("TRN2", target_bir_lowering=False, debug=False)

    xT = nc.dram_tensor("xT", [C, T], mm_dt, kind="ExternalInput")
    wq = nc.dram_tensor("wq", [C, HD], mm_dt, kind="ExternalInput")
    wk = nc.dram_tensor("wk", [C, HD], mm_dt, kind="ExternalInput")
    wv = nc.dram_tensor("wv", [C, HD], mm_dt, kind="ExternalInput")
    wo = nc.dram_tensor("wo", [HD, C], mm_dt, kind="ExternalInput")
    # bq/bk as [128, HD//128] columns (per-partition adds in qkvT layout)
    bqc = nc.dram_tensor("bqc", [128, HD // 128], F32, kind="ExternalInput")
    bkc = nc.dram_tensor("bkc", [128, HD // 128], F32, kind="ExternalInput")
    bv = nc.dram_tensor("bv", [1, HD], mm_dt, kind="ExternalInput")
    tri = nc.dram_tensor("tri", [128, 128], mm_dt, kind="ExternalInput")
    out = nc.dram_tensor("out", [T, C], F32, kind="ExternalOutput")

    with tile.TileContext(nc) as tc:
        const = tc.alloc_tile_pool(name="const", bufs=1)
        persist = tc.alloc_tile_pool(name="persist", bufs=1)
        # PSUM banks: mm [128,1024]=2x2 + mm_s [128,512]x2 + aug x2 = 8 of 8
        psum = tc.alloc_tile_pool(name="psum", bufs=2, space="PSUM")
        psum_aug = tc.alloc_tile_pool(name="psum_aug", bufs=2, space="PSUM")
        psum_s = tc.alloc_tile_pool(name="psum_s", bufs=2, space="PSUM")
        xt_pool = tc.alloc_tile_pool(name="xt", bufs=18)
        e_pool = tc.alloc_tile_pool(name="e", bufs=10)
        r_pool = tc.alloc_tile_pool(name="r", bufs=3)
        o_pool = tc.alloc_tile_pool(name="o", bufs=3)

        # --- persistent weights, loaded first via GpSimd-issued DMAs so they
        # don't serialize behind the x-chunk loads on the sync issue pipe ---
        wq_sb = persist.tile([128, KO, HD], mm_dt)
        wk_sb = persist.tile([128, KO, HD], mm_dt)
        wv_sb = persist.tile([128, KO, HD], mm_dt)
        wo_sb = persist.tile([128, HDO, C], mm_dt)
        for w_sb, w_d in ((wq_sb, wq), (wk_sb, wk), (wo_sb, wo)):
            kos = w_sb.shape[1]
            for ko in range(kos):
                nc.gpsimd.dma_start(w_sb[:, ko], w_d[ko * 128 : (ko + 1) * 128, :])

        # --- constants ---
        ones_f = const.tile([1, 512], F32)
        ones_r = const.tile([1, 512], mm_dt)
        nc.vector.memset(ones_f[:], 1.0)
        nc.vector.tensor_copy(ones_r[:], ones_f[:])
        ones_col_f = const.tile([128, 1], F32)
        nc.vector.memset(ones_col_f[:], 1.0)
        tri_sb = const.tile([128, 128], mm_dt)
        nc.sync.dma_start(tri_sb[:], tri[:, :])
        bqc_sb = const.tile([128, HD // 128], F32)
        bkc_sb = const.tile([128, HD // 128], F32)
        bv_sb = const.tile([1, HD], mm_dt)


        # --- persistent tensors (split per t-chunk so attention blocks only
        # depend on the chunks they read) ---
        qT_sb = [persist.tile([128, HDO, 512], mm_dt, name=f"qT{_t}") for _t in range(TC)]
        kT_sb = [persist.tile([128, HDO, 512], mm_dt, name=f"kT{_t}") for _t in range(TC)]
        # [tpart, ktile-in-chunk, head, d|1]
        v_sb = [persist.tile([128, 4, HPC, D + 1], mm_dt, name=f"v{_t}") for _t in range(TC)]
        for vt in v_sb:
            nc.vector.tensor_copy(
                vt[:, :, :, D], ones_col_f[:, 0:1].to_broadcast([128, 4, HPC])
            )
        attnT_sb = persist.tile([128, HDO, T], mm_dt)

        # --- phase-1 chunk emission: qT/kT/v for t in [tc4*512, tc4*512+512)
        # Emitted as a list of closures so chunks can interleave with
        # attention blocks in PE program order.
        def ph1_units(tc4):
            ts_ = slice(tc4 * 512, (tc4 + 1) * 512)
            xt = []

            def load_xt():
                for ko in range(KO):
                    t_ = xt_pool.tile([128, 512], mm_dt, tag="xt")
                    nc.sync.dma_start(t_[:], xT[ko * 128 : (ko + 1) * 128, ts_])
                    xt.append(t_)

            units = [load_xt]

            def qk_unit(w_sb, b_sb, dst, i):
                def emit():
                    cs = slice(i * 128, (i + 1) * 128)
                    ps = psum.tile([128, 1024], F32, tag="mm")
                    for ko in range(KO):
                        nc.tensor.matmul(
                            ps[:, 0:512], w_sb[:, ko, cs], xt[ko][:],
                            start=(ko == 0), stop=(ko == KO - 1),
                        )
                    if use_bias:
                        nc.vector.tensor_scalar_add(
                            dst[:, i, :], ps[:, 0:512], b_sb[:, i : i + 1]
                        )
                    else:
                        nc.vector.tensor_copy(dst[:, i, :], ps[:, 0:512])
                return emit

            def v_unit(s):
                def emit():
                    ps = psum.tile([128, 1024], F32, tag="mm")
                    for ko in range(KO):
                        nc.tensor.matmul(
                            ps[:, 0:512],
                            xt[ko][:, s * 128 : (s + 1) * 128], wv_sb[:, ko, :],
                            start=(ko == 0), stop=(not use_bias and ko == KO - 1),
                        )
                    if use_bias:
                        nc.tensor.matmul(
                            ps[:, 0:512], ones_r[0:1, 0:128], bv_sb[0:1, :],
                            start=False, stop=True, skip_group_check=True,
                        )
                    nc.vector.tensor_copy(
                        v_sb[tc4][:, s, :, 0:D],
                        ps[:, 0:512].rearrange("p (h d) -> p h d", h=HPC),
                    )
                return emit

            for i in range(HDO):
                units.append(qk_unit(wq_sb, bqc_sb, qT_sb[tc4], i))
            for i in range(HDO):
                units.append(qk_unit(wk_sb, bkc_sb, kT_sb[tc4], i))
            for s in range(4):
                units.append(v_unit(s))
            return units

        # --- attention block (h, q): uses qT chunk q, kT/v chunks <= q ---
        pending = None  # (aug, drow, pr, co, q) awaiting normalization

        def flush_norm():
            nonlocal pending
            if pending is None:
                return
            aug, drow, pr, co, q = pending
            bc = psum_s.tile([64, 512], F32, tag="mm_s", name="bc")
            nc.tensor.matmul(bc[:], ones_r[0:1, 0:64], drow[:], start=True, stop=True)
            rec = r_pool.tile([64, 512], F32, tag="rec")
            # ~4e-6 relerr, ~5x faster than exact reciprocal; denom >= ~0.04
            nc.vector.reciprocal_approx_fast(rec[:], bc[:])
            nc.vector.tensor_mul(
                attnT_sb[pr : pr + 64, co, q * 512 : (q + 1) * 512],
                aug[0:D, :], rec[:],
            )
            pending = None

        def attn_block(h, q):
            nonlocal pending
            co, pr = h // 2, (h % 2) * 64
            qTh = qT_sb[q][pr : pr + 64, co, :]
            jmax = 4 * q + 3
            aug = psum_aug.tile([D + 1, 512], F32, tag="aug")

            # build (score+exp emitter, aug emitter) steps, then emit with the
            # aug of step s-1 after the scores of step s so the PE never waits
            # on the freshest exp
            steps = []
            j = 0
            while j <= jmax:
                kTh = kT_sb[j // 4][pr : pr + 64, co, :]
                if j + 1 < 4 * q and j % 4 < 3:
                    # two full key tiles in one chunk: one 1024-wide exp
                    def mk_pair(j):
                        kTh_ = kT_sb[j // 4][pr : pr + 64, co, :]
                        e = [None]

                        def scores():
                            ps = psum.tile([128, 1024], F32, tag="mm")
                            e[0] = e_pool.tile([128, 1024], mm_dt, tag="e", name="e")
                            for u in range(2):
                                nc.tensor.matmul(
                                    ps[:, u * 512 : (u + 1) * 512],
                                    kTh_[:, (j + u) % 4 * 128 : ((j + u) % 4 + 1) * 128],
                                    qTh[:],
                                    start=True, stop=True, skip_group_check=True,
                                )
                            nc.scalar.activation(e[0][:], ps[:], EXP, scale=SCALE)

                        def augmm():
                            for u in range(2):
                                nc.tensor.matmul(
                                    aug[:], v_sb[(j + u) // 4][:, (j + u) % 4, h, :],
                                    e[0][:, u * 512 : (u + 1) * 512],
                                    start=(j + u == 0), stop=False,
                                    skip_group_check=True,
                                )
                        return scores, augmm

                    steps.append(mk_pair(j))
                    j += 2
                    continue

                def mk_single(j):
                    kTh_ = kT_sb[j // 4][pr : pr + 64, co, :]
                    diag = j >= 4 * q
                    c0 = 128 * (j - 4 * q) if diag else 0
                    ncol = 512 - c0
                    e = [None]

                    def scores():
                        ps = psum_s.tile([128, 512], F32, tag="mm_s", name="ps_s")
                        e[0] = e_pool.tile([128, 1024], mm_dt, tag="e", name="e")
                        nc.tensor.matmul(
                            ps[:, :ncol],
                            kTh_[:, j % 4 * 128 : (j % 4 + 1) * 128],
                            qTh[:, c0:512],
                            start=True, stop=True,
                        )
                        nc.scalar.activation(e[0][:, :ncol], ps[:, :ncol], EXP,
                                             scale=SCALE)
                        if diag:
                            nc.vector.tensor_mul(e[0][:, 0:128], e[0][:, 0:128],
                                                 tri_sb[:])

                    def augmm():
                        nc.tensor.matmul(
                            aug[:, c0:], v_sb[j // 4][:, j % 4, h, :],
                            e[0][:, :ncol],
                            start=(j == 0), stop=(j == jmax),
                            skip_group_check=True,
                        )
                    return scores, augmm

                steps.append(mk_single(j))
                j += 1

            LAG = 2
            for s, (scores, _) in enumerate(steps):
                scores()
                if s >= LAG:
                    steps[s - LAG][1]()
            for s in range(max(0, len(steps) - LAG), len(steps)):
                steps[s][1]()

            drow = r_pool.tile([1, 512], mm_dt, tag="drow")
            with nc.allow_low_precision(reason="softmax denom rounding"):
                nc.vector.tensor_copy(drow[:], aug[D : D + 1, :])
            flush_norm()
            pending = (aug, drow, pr, co, q)

        # --- tail unit: output projection for one t-tile (PSUM-accumulated
        # over hd tiles); ready once round tt//4 is normalized ---
        def tail_unit(tt):
            def emit():
                ps = psum.tile([128, 1024], F32, tag="mm")
                for cc in range(2):
                    for ko in range(HDO):
                        nc.tensor.matmul(
                            ps[:, cc * 512 : (cc + 1) * 512],
                            attnT_sb[:, ko, tt * 128 : (tt + 1) * 128],
                            wo_sb[:, ko, cc * 512 : (cc + 1) * 512],
                            start=(ko == 0), stop=(ko == HDO - 1),
                            skip_group_check=True,
                        )
                osb = o_pool.tile([128, 1024], F32, tag="osb")
                nc.vector.tensor_copy(osb[:], ps[:])
                nc.sync.dma_start(out[tt * 128 : (tt + 1) * 128, :], osb[:])
            return emit

        # --- emission ---
        # Round 0 starts as soon as its inputs exist: x chunk-0, qk column 0
        # and v; remaining qk columns interleave between its head pairs.
        # u0 = [load_xt, qkq0..3, qkk0..3, v0..3]
        u0 = ph1_units(0)
        u0[0]()
        for ko in range(KO):
            nc.sync.dma_start(wv_sb[:, ko], wv[ko * 128 : (ko + 1) * 128, :])
        nc.sync.dma_start(bqc_sb[:], bqc[:, :])
        nc.sync.dma_start(bkc_sb[:], bkc[:, :])
        nc.sync.dma_start(bv_sb[:], bv[:, :])
        u0[1](); u0[5]()
        for k in (9, 10, 11, 12):
            u0[k]()
        for m in range(4):
            if m > 0:
                u0[1 + m]()
                u0[5 + m]()
            attn_block(2 * m, 0)
            attn_block(2 * m + 1, 0)
        for u in ph1_units(1):  # chunk 1 (pure PE stretch before round 1)
            u()
        # rounds 1-2 interleave the next projection chunk; round 3
        # interleaves ready output-projection tiles
        for q in range(1, NQ):
            if q + 1 < TC:
                filler = ph1_units(q + 1)
                filler[0]()  # prefetch the chunk's x tiles at round start
                filler = filler[1:]
            else:
                filler = [tail_unit(tt) for tt in range(12)]
            fi = 0
            for h in range(HPC):
                attn_block(h, q)
                # spread filler units across the 8 heads
                take = (len(filler) - fi) // (HPC - h) if h < HPC else 0
                for _ in range(take):
                    filler[fi]()
                    fi += 1
            while fi < len(filler):
                filler[fi]()
                fi += 1
        flush_norm()
        for tt in range(12, NKT):
            tail_unit(tt)()

        o_pool.release()
        r_pool.release()
        e_pool.release()
        xt_pool.release()
        psum_s.release()
        psum_aug.release()
        psum.release()
        persist.release()
        const.release()

    nc.finalize()
    return nc


_NC_CACHE = {}


def _get_nc(mm_dt=None, use_bias=True):
    key = (str(mm_dt or MM_DT), use_bias)
    if key not in _NC_CACHE:
        _NC_CACHE[key] = build_nc(mm_dt, use_bias=use_bias)
    return _NC_CACHE[key]


def make_in_maps(x, Wqkv, bqkv, Wo, mm_dt=None):
    mdt = _np_of(mm_dt or MM_DT)
    x = np.asarray(x, dtype=np.float32)
    Wqkv = np.asarray(Wqkv, dtype=np.float32)
    bqkv = np.asarray(bqkv, dtype=np.float32)
    Wo = np.asarray(Wo, dtype=np.float32)

    w3 = Wqkv.reshape(C, 3, H, D)
    b3 = bqkv.reshape(3, H, D)
    wo4 = Wo.reshape(H, D, C)
    tri = np.triu(np.ones((128, 128), dtype=np.float32))

    in_maps = []
    for c in range(N_CORES):
        b, g = c // 2, c % 2
        hs = slice(g * HPC, (g + 1) * HPC)
        bq = b3[0, hs].reshape(HD)
        bk = b3[1, hs].reshape(HD)
        in_maps.append({
            "xT": np.ascontiguousarray(x[b].T).astype(mdt),
            "wq": np.ascontiguousarray(w3[:, 0, hs, :].reshape(C, HD)).astype(mdt),
            "wk": np.ascontiguousarray(w3[:, 1, hs, :].reshape(C, HD)).astype(mdt),
            "wv": np.ascontiguousarray(w3[:, 2, hs, :].reshape(C, HD)).astype(mdt),
            "wo": np.ascontiguousarray(wo4[hs].reshape(HD, C)).astype(mdt),
            "bqc": np.ascontiguousarray(bq.reshape(HD // 128, 128).T).astype(np.float32),
            "bkc": np.ascontiguousarray(bk.reshape(HD // 128, 128).T).astype(np.float32),
            "bv": b3[2, hs].reshape(1, HD).astype(mdt),
            "tri": tri.astype(mdt),
        })
    return in_maps


def run(x, Wqkv, bqkv, Wo, bo, mm_dt=None, **spmd_kwargs):
    use_bias = bool(np.any(np.asarray(bqkv)))
    nc = _get_nc(mm_dt, use_bias=use_bias)
    in_maps = make_in_maps(x, Wqkv, bqkv, Wo, mm_dt=mm_dt)
    res = run_bass_kernel_spmd(nc, in_maps, core_ids=list(range(N_CORES)),
                               **spmd_kwargs)
    bo = np.asarray(bo, dtype=np.float32)
    out = np.empty((B, T, C), dtype=np.float32)
    for b in range(B):
        out[b] = res.results[2 * b]["out"] + res.results[2 * b + 1]["out"] + bo
    return out, res


def kernel(x, Wqkv, bqkv, Wo, bo):
    out, _ = run(x, Wqkv, bqkv, Wo, bo)
    return out
